# revision 32
# baseline (speedup 1.0000x reference)
"""Conv2d(32->64,3x3,valid) + bias + Mish + BatchNorm(batch stats) on trn2 x8.

Strategy: data-parallel over N (2 images/core). Conv via 3 accumulating
matmuls per 2-output-row block (K=(c_in,4 rows)=128, M=(c_out,row parity)=128).
Mish computed from sigmoid: w = sigmoid(conv+bias), t = tanh(softplus) =
w(2-w)/(1+(1-w)^2), reciprocal via custom DVE op (poly seed + 1 Newton step).
Mish kept SBUF-resident in fp16; BN stats (sum/sumsq) fused into the
elementwise ops via accum_out, AllReduce'd across the 8 cores; second pass
normalizes and writes fp32 output.
"""

import numpy as np

N, C_IN, H, W = 16, 32, 256, 256
C_OUT, KK = 64, 3
HO = WO = 254
N_CORES = 8
NL = N // N_CORES          # images per core
NBLK = HO // 2             # 127 2-row blocks
NGRP = 32                  # groups of <=4 blocks (psum tile = 4 banks)
GROUPS = [(g, 4 if g < NGRP - 1 else NBLK - 4 * (NGRP - 1)) for g in range(NGRP)]
EPS = 1e-5
COUNT = float(N * HO * WO)

# deg-2 minimax seed for 1/(1+s) on s in [0,1]; one Newton step -> ~1e-4 rel
RC0, RC1, RC2 = 0.98986264, -0.80798751, 0.32319316

_CACHE = {}


def _register_custom_ops():
    if "ops" in _CACHE:
        return _CACHE["ops"]
    from concourse.dve_spec import Spec, Src0, Src1, C0, C1, C2, One, sq, lower
    from concourse.dve_spec import _has_src1 as has_src1
    from concourse import dve_ops
    from concourse.dve_ops import DveOp, OPS, CUSTOM_DVE_SPECS, _SUB_OPCODE_FOR_NAME
    from concourse.dve_uop import DveOpSpec
    from concourse.dve_table_gen import dve_ver_for

    def _make(name, spec):
        for op in OPS:
            if op.name == name:
                return op
        _SUB_OPCODE_FOR_NAME[name] = max(_SUB_OPCODE_FOR_NAME.values()) + 1
        shas = {}
        for ver in ("v3", "v4"):
            try:
                uops = lower(spec, ver=ver)
                shas[ver] = DveOpSpec(
                    name=name,
                    opcode=_SUB_OPCODE_FOR_NAME[name],
                    uops=uops,
                    rd1_en=has_src1(spec),
                ).sha(ver)
            except Exception:
                pass
        op = DveOp(name, spec, subdim=False, uops_sha=shas,
                   perf_en=({"v3": True, "v4": True} if name == "NEGT_ANT" else {}))
        OPS.append(op)
        CUSTOM_DVE_SPECS[name] = spec
        return op

    TWO = One + One
    # opA: in0 = q = sigmoid(-a); v = q^2; y0 = C0 + v*(C1 + C2*v) ~ 1/(1+v)
    _v = sq(Src0)
    seedq = Spec(
        body=C0 + _v * (C1 + C2 * _v),
        reference=lambda in0, in1, c0, c1, c2: (
            (lambda v: c0 + v * (c1 + c2 * v))(in0 * in0)
        ),
    )
    # opB: in0 = q, in1 = y0: v=q^2; d=1+v; y1=y0*(2-d*y0); mt=(v-1)*y1 = -t
    _vb = sq(Src0)
    mtq = Spec(
        body=((_vb - One) * Src1) * (TWO - (One + _vb) * Src1),
        reference=lambda in0, in1, c0, c1, c2: (
            (lambda v: (v - 1.0) * in1 * (2.0 - (1.0 + v) * in1))(in0 * in0)
        ),
    )
    ops = (_make("SEEDQ_ANT", seedq), _make("MTQ_ANT", mtq))
    _CACHE["ops"] = ops
    return ops


def _build():
    if "nc" in _CACHE:
        return _CACHE["nc"]
    import concourse.bacc as bacc
    import concourse.mybir as mybir
    import concourse.tile as tile
    import concourse.bass as bass

    SEEDQ, MTQ = _register_custom_ops()

    dt = mybir.dt
    AFT = mybir.ActivationFunctionType
    ALU = mybir.AluOpType
    AXL = mybir.AxisListType

    nc = bacc.Bacc("TRN2", target_bir_lowering=False, debug=False, num_devices=N_CORES)

    x_d = nc.dram_tensor("xe", [C_IN, 4, NBLK, NL, W], dt.float16, kind="ExternalInput")
    wt_d = nc.dram_tensor("wt", [KK, 128, 128], dt.float16, kind="ExternalInput")
    bias_d = nc.dram_tensor("bias128", [128, 1], dt.float32, kind="ExternalInput")
    bnw_d = nc.dram_tensor("bnw", [64, 1], dt.float32, kind="ExternalInput")
    bnb_d = nc.dram_tensor("bnb", [64, 1], dt.float32, kind="ExternalInput")
    y_d = nc.dram_tensor("yt", [2, C_OUT, NBLK, NL, WO], dt.float16, kind="ExternalOutput")

    with tile.TileContext(nc) as tc:
        with (
            tc.tile_pool(name="const", bufs=1) as cpool,
            tc.tile_pool(name="mish", bufs=1) as mpool,
            tc.tile_pool(name="xg", bufs=3) as xpool,
            tc.tile_pool(name="chain", bufs=9) as fpool,
            tc.tile_pool(name="stage", bufs=4) as stpool,
            tc.tile_pool(name="psum", bufs=2, space="PSUM") as ppool,
            tc.tile_pool(name="dram", bufs=1, space="DRAM") as dpool,
        ):
            # constants
            wts = cpool.tile([128, KK * 128], dt.float16)
            for kw in range(KK):
                nc.sync.dma_start(wts[:, kw * 128:(kw + 1) * 128], wt_d[kw, :, :])
            bias_t = cpool.tile([128, 1], dt.float32)
            nc.sync.dma_start(bias_t[:, :], bias_d[:, :])
            nbias_t = cpool.tile([128, 1], dt.float32)
            nc.vector.tensor_scalar_mul(nbias_t[:, :], bias_t[:, :], -1.0)
            eps_t = cpool.tile([64, 1], dt.float32)
            nc.vector.memset(eps_t[:, :], EPS)
            bnw_t = cpool.tile([64, 1], dt.float32)
            nc.sync.dma_start(bnw_t[:, :], bnw_d[:, :])
            bnb_t = cpool.tile([64, 1], dt.float32)
            nc.sync.dma_start(bnb_t[:, :], bnb_d[:, :])

            mish_res = mpool.tile([128, NBLK * 2 * WO], dt.float16)
            stat_ns = cpool.tile([128, NGRP], dt.float32)
            stat_sq = cpool.tile([128, NGRP], dt.float32)

            # ---------------- pass 1: conv + mish + stats ----------------
            for g, nb in GROUPS:
                ncols = nb * 512
                xg = xpool.tile([128, 2048], dt.float16, tag="xg")
                nc.sync.dma_start(
                    xg[:, :nb * 512],
                    x_d[:, :, 4 * g: 4 * g + nb, :, :],
                )
                ps = ppool.tile([128, 2048], dt.float32, tag="ps")
                for b in range(nb):
                    for kw in range(KK):
                        nc.tensor.matmul(
                            ps[:, b * 512: b * 512 + 510],
                            lhsT=wts[:, kw * 128:(kw + 1) * 128],
                            rhs=xg[:, b * 512 + kw: b * 512 + kw + 510],
                            start=(kw == 0),
                            stop=(kw == KK - 1),
                        )
                av = fpool.tile([128, 2048], dt.float16, tag="chain")
                nc.scalar.activation(
                    av[:, :ncols], ps[:, :ncols], AFT.Identity,
                    bias=bias_t[:, :],
                )
                qq = fpool.tile([128, 2048], dt.float16, tag="chain")
                nc.scalar.activation(
                    qq[:, :ncols], ps[:, :ncols], AFT.Sigmoid,
                    bias=nbias_t[:, :], scale=-1.0,
                )
                y0 = fpool.tile([128, 2048], dt.float16, tag="chain")
                nc.vector._custom_dve(
                    SEEDQ, out=y0[:, :ncols], in0=qq[:, :ncols],
                    s0=RC0, s1=RC1, imm2=RC2,
                )
                mt = fpool.tile([128, 2048], dt.float16, tag="chain")
                nc.vector._custom_dve(
                    MTQ, out=mt[:, :ncols], in0=qq[:, :ncols], in1=y0[:, :ncols],
                )
                base = g * 4 * 2 * WO
                mslice = mish_res[:, base: base + nb * 2 * WO]
                a_v = av[:, :ncols].rearrange("p (b n v) -> p b n v", b=nb, v=256)[
                    :, :, :, 0:WO
                ]
                mt_v = mt[:, :ncols].rearrange("p (b n v) -> p b n v", b=nb, v=256)[
                    :, :, :, 0:WO
                ]
                m_out = mslice.rearrange("p (b n w) -> p b n w", b=nb, w=WO)
                # m = -mish = (conv + bias) * (-t); accum = sum(-mish)
                nc.vector.scalar_tensor_tensor(
                    out=m_out, in0=a_v, scalar=0.0, in1=mt_v,
                    op0=ALU.add, op1=ALU.mult,
                    accum_out=stat_ns[:, g:g + 1],
                )
                nc.scalar.activation(
                    y0[:, :nb * 2 * WO], mslice, AFT.Square,
                    accum_out=stat_sq[:, g:g + 1],
                )

            # ---------------- stats: local reduce + AllReduce + scale/shift ----
            red = cpool.tile([128, 2], dt.float32)
            nc.vector.reduce_sum(red[:, 0:1], stat_ns[:, :], axis=AXL.X)
            nc.vector.reduce_sum(red[:, 1:2], stat_sq[:, :], axis=AXL.X)
            cc_in = dpool.tile([128, 2], dt.float32)
            cc_out = dpool.tile([128, 2], dt.float32)
            nc.sync.dma_start(cc_in[:, :], red[:, :])
            nc.gpsimd.collective_compute(
                "AllReduce",
                ALU.add,
                replica_groups=[list(range(N_CORES))],
                ins=[cc_in.opt()],
                outs=[cc_out.opt()],
            )
            ar = cpool.tile([128, 2], dt.float32)
            nc.sync.dma_start(ar[:, :], cc_out[:, :])
            fold = cpool.tile([64, 2], dt.float32)
            nc.sync.dma_start(fold[:, :], ar[64:128, :])
            tot = cpool.tile([64, 2], dt.float32)
            nc.vector.tensor_tensor(tot[:, :], ar[0:64, :], fold[:, :], op=ALU.add)
            mstats = cpool.tile([64, 2], dt.float32)  # [:,0] = -mean, [:,1] = E[m^2]
            nc.vector.tensor_scalar_mul(mstats[:, :], tot[:, :], 1.0 / COUNT)
            nvar = cpool.tile([64, 1], dt.float32)  # mean^2 - E[m^2] = -var
            nc.vector.scalar_tensor_tensor(
                out=nvar[:, :], in0=mstats[:, 0:1], scalar=mstats[:, 0:1],
                in1=mstats[:, 1:2], op0=ALU.mult, op1=ALU.subtract,
            )
            std = cpool.tile([64, 1], dt.float32)  # sqrt(-(-var) + eps)
            nc.scalar.activation(std[:, :], nvar[:, :], AFT.Sqrt, bias=eps_t[:, :], scale=-1.0)
            istd = cpool.tile([64, 1], dt.float32)
            nc.vector.reciprocal(istd[:, :], std[:, :])
            scl = cpool.tile([64, 1], dt.float32)
            nc.vector.tensor_scalar(
                out=scl[:, :], in0=istd[:, :], scalar1=bnw_t[:, :], scalar2=None,
                op0=ALU.mult,
            )
            nscl = cpool.tile([64, 1], dt.float32)
            nc.vector.tensor_scalar_mul(nscl[:, :], scl[:, :], -1.0)
            shf = cpool.tile([64, 1], dt.float32)  # (-mean)*scl + bnb
            nc.vector.scalar_tensor_tensor(
                out=shf[:, :], in0=scl[:, :], scalar=mstats[:, 0:1],
                in1=bnb_t[:, :], op0=ALU.mult, op1=ALU.add,
            )
            ns128 = cpool.tile([128, 1], dt.float32)
            nc.sync.dma_start(ns128[0:64, :], nscl[:, :])
            nc.sync.dma_start(ns128[64:128, :], nscl[:, :])
            sh128 = cpool.tile([128, 1], dt.float32)
            nc.sync.dma_start(sh128[0:64, :], shf[:, :])
            nc.sync.dma_start(sh128[64:128, :], shf[:, :])

            # ---------------- pass 2: normalize + write out ----------------
            j = 0
            while j < NBLK:
                nb2 = min(4, NBLK - j)
                cols = nb2 * 2 * WO
                st = stpool.tile([128, 4 * 2 * WO], dt.float16, tag="st")
                msl = mish_res[:, j * 2 * WO: j * 2 * WO + cols]
                # out = m * (-scale) + shift  (m = -mish)
                nc.vector.tensor_scalar(
                    out=st[:, :cols], in0=msl,
                    scalar1=ns128[:, :], scalar2=sh128[:, :],
                    op0=ALU.mult, op1=ALU.add,
                )
                nc.sync.dma_start(
                    y_d[:, :, j: j + nb2, :, :],
                    st[:, :cols],
                )
                j += nb2

    nc.compile()
    _CACHE["nc"] = nc
    return nc


def _prep_inputs(x, weight, bias, bn_weight, bn_bias):
    # lhsT[kw][(ci*4+r), (parity*64+co)] = W[co, ci, r-parity, kw]
    w = np.asarray(weight, dtype=np.float32)
    lhsT = np.zeros((KK, 32, 4, 2, 64), dtype=np.float32)
    for r in range(4):
        for p in range(2):
            kh = r - p
            if 0 <= kh <= 2:
                # w[co, ci, kh, kw] -> lhsT[kw, ci, r, p, co]
                lhsT[:, :, r, p, :] = np.transpose(w[:, :, kh, :], (2, 1, 0))
    wt = lhsT.reshape(KK, 128, 128).astype(np.float16)

    bias128 = np.tile(np.asarray(bias, dtype=np.float32), 2).reshape(128, 1)
    bnw64 = np.asarray(bn_weight, dtype=np.float32).reshape(64, 1)
    bnb64 = np.asarray(bn_bias, dtype=np.float32).reshape(64, 1)

    x16 = np.asarray(x, dtype=np.float16)
    in_maps = []
    for c in range(N_CORES):
        xs = x16[c * NL:(c + 1) * NL]            # [NL, C_IN, H, W]
        xt = xs.transpose(1, 2, 0, 3)            # [C_IN, H, NL, W]
        xe = np.empty((C_IN, 4, NBLK, NL, W), dtype=np.float16)
        for r in range(4):
            xe[:, r] = xt[:, r: r + 2 * NBLK: 2]  # rows 2b+r
        in_maps.append({
            "xe": xe,
            "wt": wt,
            "bias128": bias128,
            "bnw": bnw64,
            "bnb": bnb64,
        })
    return in_maps


def kernel(x, weight, bias, bn_weight, bn_bias):
    from concourse import bass_utils

    nc = _build()
    in_maps = _prep_inputs(x, weight, bias, bn_weight, bn_bias)
    res = bass_utils.run_bass_kernel_spmd(nc, in_maps, core_ids=list(range(N_CORES)))
    return _postprocess(res.results)


def _postprocess(results):
    outs = []
    for r in results:
        yt = r["yt"]  # [2, C_OUT, NBLK, NL, WO] = (parity, c, b, n, w)
        y = yt.astype(np.float32).transpose(3, 1, 2, 0, 4).reshape(NL, C_OUT, HO, WO)
        outs.append(y)
    return np.ascontiguousarray(np.concatenate(outs, axis=0), dtype=np.float32)
